# revision 3
# baseline (speedup 1.0000x reference)
"""Causal self-attention (B=2, L=2048, H=16, D=64) head-sharded over 8 TRN2 cores.

v4: fp16, generator-scheduled single-pass program per core (heads {2c, 2c+1}):
  - fp16 everywhere except psum accumulation (f32).
  - emission is driven round-by-round: attention rounds yield between the
    S^T matmul/exp and the P@V consume; a worker queue of projection / Wo
    generators is pumped into each gap so PE never idles on the ACT exp.
  - rmsnorm: stats matmul with block-diagonal 1/D lhsT broadcasts var to all
    128 partitions; rstd = exp(-0.5*ln(var+eps)); gammas folded into 4 rope
    tables host-side; An/Bn/sq read the psum directly (no staging copies).
  - attention: S^T per 128-k-block; exp with constant bias; P@V token-major
    (out[q, d] per 128-q-block with ones-column rowsum); normalization via
    per-partition tensor_scalar; transpose to d-major via identity matmul.
  - Wo row-shard: each core emits a full-size fp16 partial out^T; host sums.
"""

from collections import deque

import numpy as np

import concourse.bacc as bacc
import concourse.bass as bass
import concourse.mybir as mybir
import concourse.tile as tile
from concourse import bass_utils
from concourse.hw_specs import get_activation_tables

F32 = mybir.dt.float32
FP16 = mybir.dt.float16

CFG = dict(B=2, L=2048, H=16, D=64, EPS=1e-6)
N_CORES = 8

TOKCH = 512   # token chunk for QKV projection
QCH = 512     # attention q chunk
KBLK = 128    # attention k block

ACT_TABLE = "natural_log_exp_and_others"


def _pin_act_table(nc):
    try:
        tabs = get_activation_tables(nc.m.arch)
    except Exception:
        return
    if ACT_TABLE in tabs:
        mine = {mybir.ActivationFunctionType.Exp,
                mybir.ActivationFunctionType.Ln,
                mybir.ActivationFunctionType.Copy}
        for k in tabs:
            if k != ACT_TABLE:
                tabs[k] = tabs[k] - mine


def build_program(cfg, c_bias, debug=False):
    B, L, H, D = cfg["B"], cfg["L"], cfg["H"], cfg["D"]
    HID = H * D
    BT = B * L
    NHID = HID // 128           # hidden (contraction) chunks (8)
    NQC = L // QCH              # q chunks per batch (4)
    NKB = L // KBLK             # k blocks per batch (16)
    CPB = L // TOKCH            # chunks per batch (4)
    scale = 1.0 / float(np.sqrt(D))
    Exp = mybir.ActivationFunctionType.Exp
    Ln = mybir.ActivationFunctionType.Ln

    nc = bacc.Bacc("TRN2", target_bir_lowering=False, debug=False,
                   num_devices=N_CORES)
    _pin_act_table(nc)

    xT = nc.dram_tensor("xT", [HID, BT], FP16, kind="ExternalInput").ap()
    wqkv = nc.dram_tensor("wqkv", [HID, 384], FP16, kind="ExternalInput").ap()
    wo = nc.dram_tensor("wo", [128, HID], FP16, kind="ExternalInput").ap()
    ropet_d = nc.dram_tensor("ropet_d", [128, 4, BT], FP16,
                             kind="ExternalInput").ap()
    tri_d = nc.dram_tensor("tri_d", [128, 128], FP16, kind="ExternalInput").ap()
    M_d = nc.dram_tensor("M_d", [128, 128], FP16, kind="ExternalInput").ap()
    id_d = nc.dram_tensor("id_d", [128, 128], FP16, kind="ExternalInput").ap()
    outT = nc.dram_tensor("outT", [HID, BT], FP16, kind="ExternalOutput").ap()

    with tile.TileContext(nc) as tc:
        with tc.tile_pool(name="const", bufs=1) as const, \
             tc.tile_pool(name="big", bufs=1) as big, \
             tc.tile_pool(name="p1", bufs=2) as p1, \
             tc.tile_pool(name="stps", bufs=2, space="PSUM") as stps, \
             tc.tile_pool(name="auxps", bufs=2, space="PSUM") as auxps, \
             tc.tile_pool(name="pvps", bufs=1, space="PSUM") as pvps:

            w_sb = const.tile([128, NHID, 384], FP16)
            ropet = const.tile([128, 4, BT], FP16)   # csA, snA, csB, snB
            wo_sb = const.tile([128, HID], FP16)
            tri_sb = const.tile([128, 128], FP16)
            M_sb = const.tile([128, 128], FP16)
            id_sb = const.tile([128, 128], FP16)
            eps_sb = const.tile([128, 1], F32)
            cb_sb = const.tile([128, 1], F32)
            zcol = const.tile([1, 128], FP16)
            zrow = const.tile([1, 512], FP16)

            QK = big.tile([128, 2 * BT], FP16)     # [:, 0:BT]=Q^T, [BT:]=K^T
            Vall = big.tile([128, NKB * B, 130], FP16)
            attnT = [big.tile([128, L], FP16, name=f"attnT{b}")
                     for b in range(B)]

            def emit_consts():
                nc.vector.memset(eps_sb, float(cfg["EPS"]))
                nc.vector.memset(cb_sb, -float(c_bias))
                nc.gpsimd.memset(Vall[:, :, 64:65], 1.0)
                nc.gpsimd.memset(Vall[:, :, 129:130], 1.0)
                nc.gpsimd.memset(zcol, 0.0)
                nc.gpsimd.memset(zrow, 0.0)
                nc.sync.dma_start(out=M_sb, in_=M_d)
                nc.sync.dma_start(out=id_sb, in_=id_d)
                nc.sync.dma_start(out=tri_sb, in_=tri_d)

            def emit_xload_split(t):
                x_sb = p1.tile([128, NHID, 2 * TOKCH], FP16, tag="x", bufs=2,
                               name="x_sb")
                for k in range(NHID):
                    nc.sync.dma_start(
                        out=x_sb[:, k, 0:TOKCH],
                        in_=xT[128 * k:128 * (k + 1), t * TOKCH:(t + 1) * TOKCH])
                return x_sb

            def emit_xload_second(x_sb, t):
                nc.sync.dma_start(
                    out=x_sb[:, :, TOKCH:2 * TOKCH],
                    in_=xT[:, t * TOKCH:(t + 1) * TOKCH].rearrange(
                        "(k p) t -> p k t", p=128))

            def emit_xload(pair):
                x_sb = p1.tile([128, NHID, 2 * TOKCH], FP16, tag="x", bufs=2,
                               name="x_sb")
                span = slice(pair * 2 * TOKCH, (pair + 1) * 2 * TOKCH)
                nc.sync.dma_start(
                    out=x_sb,
                    in_=xT[:, span].rearrange("(k p) t -> p k t", p=128))
                return x_sb

            t1big = {}
            t2big = {}
            x_tiles = {}

            def chunk_gen(t):
                """Projection + rmsnorm + rope for token chunk t.
                Yields PE-row counts between steps."""
                if t % 2 == 0 and t >= 4:
                    x_tiles[t // 2] = emit_xload(t // 2)
                x_sb = x_tiles[t // 2]
                xoff = (t % 2) * TOKCH
                xs = slice(xoff, xoff + TOKCH)
                blk0 = t * (TOKCH // KBLK)
                A_ps = auxps.tile([128, TOKCH], F32, tag="aux", name="A_ps")
                B_ps = auxps.tile([128, TOKCH], F32, tag="aux", name="B_ps")
                for m, ps in enumerate((A_ps, B_ps)):
                    for k in range(NHID):
                        nc.tensor.matmul(
                            ps, w_sb[:, k, 128 * m:128 * (m + 1)],
                            x_sb[:, k, xs],
                            start=(k == 0), stop=(k == NHID - 1),
                            skip_group_check=True)
                        if k == 3:
                            yield 4 * TOKCH
                    yield 4 * TOKCH
                # stage A/B to sbuf (frees the aux psum slots), then stats
                A_sb = p1.tile([128, TOKCH], FP16, tag="A", name="A_sb")
                B_sb = p1.tile([128, TOKCH], FP16, tag="B", name="B_sb")
                nc.vector.tensor_copy(A_sb, A_ps)
                nc.vector.tensor_copy(B_sb, B_ps)
                sqA = p1.tile([128, TOKCH], FP16, tag="sqA", name="sqA")
                sqB = p1.tile([128, TOKCH], FP16, tag="sqB", name="sqB")
                An = p1.tile([128, TOKCH], FP16, tag="An", name="An")
                Bn = p1.tile([128, TOKCH], FP16, tag="Bn", name="Bn")
                nc.vector.tensor_mul(sqA, A_sb, A_sb)
                nc.gpsimd.tensor_mul(sqB, B_sb, B_sb)
                var_ps = auxps.tile([128, TOKCH], F32, tag="aux", name="var_ps")
                nc.tensor.matmul(var_ps, M_sb, sqA, start=True, stop=False,
                                 skip_group_check=True)
                nc.tensor.matmul(var_ps, M_sb, sqB, start=False, stop=True,
                                 skip_group_check=True)
                lnv = p1.tile([128, TOKCH], F32, tag="lnv", name="lnv")
                nc.scalar.activation(lnv, var_ps, Ln, bias=eps_sb)
                rstd = p1.tile([128, TOKCH], FP16, tag="rstd", name="rstd")
                nc.scalar.activation(rstd, lnv, Exp, scale=-0.5)
                yield 2 * TOKCH
                # V projection (token-major)
                v_ps = auxps.tile([128, 4, 128], F32, tag="aux", name="v_ps")
                for blk in range(4):
                    for k in range(NHID):
                        nc.tensor.matmul(
                            v_ps[:, blk, :],
                            x_sb[:, k, xoff + 128 * blk: xoff + 128 * (blk + 1)],
                            w_sb[:, k, 256:384],
                            start=(k == 0), stop=(k == NHID - 1),
                            skip_group_check=True)
                    if blk == 1:
                        yield 2 * 8 * 128
                nc.vector.tensor_copy(
                    Vall[:, blk0:blk0 + 4, :].rearrange(
                        "p c (a d) -> p c a d", a=2)[:, :, :, 0:64],
                    v_ps.rearrange("p c (a d) -> p c a d", a=2))
                nc.vector.tensor_mul(An, A_sb, rstd)
                nc.gpsimd.tensor_mul(Bn, B_sb, rstd)
                yield 2 * 8 * 128
                # rope
                pair = t // 2
                if pair not in t1big:
                    t1big[pair] = p1.tile([128, 2 * TOKCH], FP16, tag="t1",
                                          bufs=2, name="t1big")
                    t2big[pair] = p1.tile([128, 2 * TOKCH], FP16, tag="t2",
                                          bufs=2, name="t2big")
                t1 = t1big[pair][:, xs]
                t2 = t2big[pair][:, xs]
                ts_ = slice(t * TOKCH, (t + 1) * TOKCH)
                csA = ropet[:, 0, ts_]
                snA = ropet[:, 1, ts_]
                csB = ropet[:, 2, ts_]
                snB = ropet[:, 3, ts_]
                ta = p1.tile([128, TOKCH], FP16, tag="ta", name="ta")
                tb = p1.tile([128, TOKCH], FP16, tag="tb", name="tb")
                nc.vector.tensor_mul(ta, An, csA)
                nc.gpsimd.tensor_mul(tb, Bn, snB)
                nc.gpsimd.tensor_sub(t1, ta, tb)
                tc_ = p1.tile([128, TOKCH], FP16, tag="tc", name="tc_")
                td = p1.tile([128, TOKCH], FP16, tag="td", name="td")
                nc.vector.tensor_mul(tc_, An, snA)
                nc.gpsimd.tensor_mul(td, Bn, csB)
                nc.gpsimd.tensor_add(t2, tc_, td)
                if t % 2 == 1:
                    base = pair * 2 * TOKCH
                    for src, half in ((t1big[pair], 0), (t2big[pair], 1)):
                        for g in range(4):      # [q1h0|q1h1|k1h0|k1h1]
                            qk = g // 2
                            h = g % 2
                            dst = QK[64 * h + 32 * half:64 * h + 32 * (half + 1),
                                     qk * BT + base: qk * BT + base + 2 * TOKCH]
                            nc.sync.dma_start(
                                out=dst, in_=src[32 * g:32 * (g + 1), :])
                yield 0

            def att_gen(b, j):
                """Attention for q chunk j of batch b; yields between the
                st/exp emit and the (delayed one round) pv emit."""
                nkb = 4 * (j + 1)
                qbase = b * L + j * QCH
                pv = pvps.tile([128, 4, 2, 128], F32, tag="pv", name="pv")
                # zero the pv banks so every (qq, h) accumulation group can
                # use pure accumulates (start=False): psum start=True zeroes
                # whole 2KB regions, so per-group starts within a shared
                # bank would clobber siblings. Each group is padded to 128
                # f32 so no matmul output crosses a bank boundary.
                pvf = pv.rearrange("p a b c -> p (a b c)")
                nc.tensor.matmul(pvf[:, 0:512], zcol, zrow,
                                 start=True, stop=False, skip_group_check=True)
                nc.tensor.matmul(pvf[:, 512:1024], zcol, zrow,
                                 start=True, stop=False, skip_group_check=True)
                fin_q = deque()

                def consume(i, pexp):
                    s_off = KBLK * i - QCH * j
                    diag = s_off >= 0
                    qq0 = max(0, i - 4 * j)
                    for qq in list(range(qq0 + 1, 4)) + [qq0]:
                        qsl = slice(128 * qq, 128 * (qq + 1))
                        qb = 4 * j + qq
                        for h in range(2):
                            nc.tensor.matmul(
                                pv[:, qq, h, 0:65],
                                pexp[:, h, qsl],
                                Vall[:, b * NKB + i, 65 * h:65 * (h + 1)],
                                start=False, stop=(i == qb),
                                skip_group_check=True)
                    if diag:
                        qq = i - 4 * j
                        qb = 4 * j + qq
                        rs = p1.tile([128, 2, 1], F32, tag="rs", bufs=8,
                                     name="rs")
                        with nc.allow_low_precision(reason="rowsum recip"):
                            nc.vector.reciprocal(rs, pv[:, qq, :, 64:65])
                        attn_sb = p1.tile([128, 128], FP16, tag="attn",
                                          bufs=8, name="attn_sb")
                        nc.vector.tensor_scalar_mul(
                            attn_sb[:, 0:64], pv[:, qq, 0, 0:64], rs[:, 0, :])
                        nc.vector.tensor_scalar_mul(
                            attn_sb[:, 64:128], pv[:, qq, 1, 0:64], rs[:, 1, :])
                        tr = auxps.tile([128, 128], F32, tag="aux", name="tr")
                        nc.tensor.matmul(tr, attn_sb, id_sb,
                                         start=True, stop=True,
                                         skip_group_check=True)
                        nc.vector.tensor_copy(
                            attnT[b][:, 128 * qb:128 * (qb + 1)], tr)

                pending = deque()
                for i in range(nkb):
                    s_off = KBLK * i - QCH * j
                    diag = s_off >= 0
                    sp = slice(max(s_off, 0), QCH)
                    st = stps.tile([128, 2, QCH], F32, tag="st", name="st")
                    for h in range(2):
                        nc.tensor.matmul(
                            st[:, h, sp],
                            QK[64 * h:64 * (h + 1),
                               BT + b * L + KBLK * i: BT + b * L + KBLK * (i + 1)],
                            QK[64 * h:64 * (h + 1), qbase + sp.start:qbase + QCH],
                            start=True, stop=not diag, skip_group_check=True)
                        if diag:
                            # add -C to the strict upper triangle of the
                            # diagonal 128-block so exp gives ~0 there
                            nc.tensor.matmul(
                                st[:, h, sp.start:sp.start + KBLK],
                                id_sb, tri_sb,
                                start=False, stop=True, skip_group_check=True)
                    pexp = p1.tile([128, 2, QCH], FP16, tag="pexp", bufs=9,
                                   name="pexp")
                    nc.scalar.activation(
                        pexp[:, :, sp], st[:, :, sp],
                        Exp, bias=cb_sb, scale=scale)
                    yield
                    pending.append((i, pexp))
                    if len(pending) > 7:
                        consume(*pending.popleft())
                    yield
                while pending:
                    consume(*pending.popleft())

            wo_cp = [0]
            ob1 = {}

            def wo_cols_gen(b, jj, use_act=True):     # jj-major
                js = slice(jj * QCH, (jj + 1) * QCH)
                jl = slice((jj % 2) * QCH, (jj % 2 + 1) * QCH)
                for o in range(NHID):
                    if (b, o) not in ob1:
                        ob1[(b, o)] = p1.tile([128, 2 * QCH], FP16,
                                              tag=f"ob{b}", bufs=NHID,
                                              name=f"ob{b}_{o}")
                    ops = auxps.tile([128, QCH], F32, tag="aux", name="ops")
                    nc.tensor.matmul(ops, wo_sb[:, 128 * o:128 * (o + 1)],
                                     attnT[b][:, js],
                                     start=True, stop=True,
                                     skip_group_check=True)
                    if use_act and wo_cp[0] % 3 == 2:
                        nc.scalar.copy(ob1[(b, o)][:, jl], ops)
                    else:
                        nc.vector.tensor_copy(ob1[(b, o)][:, jl], ops)
                    wo_cp[0] += 1
                    yield QCH
                if jj % 2 == 1:               # store a half per o
                    hs = slice((jj - 1) * QCH, (jj + 1) * QCH)
                    for o in range(NHID):
                        nc.sync.dma_start(
                            out=outT[128 * o:128 * (o + 1),
                                     b * L + hs.start:b * L + hs.stop],
                            in_=ob1[(b, o)])

            # ---------- scheduler ----------
            work = deque()
            done_chunks = set()

            def wrap_chunk(t):
                def g():
                    for r in chunk_gen(t):
                        yield r
                    done_chunks.add(t)
                return g()

            def pump(rows):
                while rows > 0 and work:
                    try:
                        rows -= next(work[0])
                    except StopIteration:
                        work.popleft()

            def flush_chunk(t):
                while t not in done_chunks and work:
                    try:
                        next(work[0])
                    except StopIteration:
                        work.popleft()

            def flush_all():
                while work:
                    try:
                        next(work[0])
                    except StopIteration:
                        work.popleft()

            QUOTA = 500          # PE rows pumped per attention yield

            # ---------- emission schedule ----------
            for k in range(NHID):
                nc.sync.dma_start(out=w_sb[:, k, :],
                                  in_=wqkv[128 * k:128 * (k + 1), :])
            x_tiles[0] = emit_xload_split(0)
            emit_consts()
            nc.sync.dma_start(out=ropet[:, :, 0:2 * TOKCH],
                              in_=ropet_d[:, :, 0:2 * TOKCH])
            emit_xload_second(x_tiles[0], 1)
            x_tiles[1] = emit_xload(1)        # prefetch chunks 2-3
            for _ in chunk_gen(0):
                pass
            nc.sync.dma_start(out=ropet[:, :, 2 * TOKCH:],
                              in_=ropet_d[:, :, 2 * TOKCH:])
            for _ in chunk_gen(1):
                pass
            nc.sync.dma_start(out=wo_sb, in_=wo)
            work.extend([wrap_chunk(2), wrap_chunk(3), wrap_chunk(4),
                         wrap_chunk(5), wrap_chunk(6), wrap_chunk(7)])
            for j in range(NQC):                      # b0 attention
                if j == 1:
                    flush_chunk(3)    # pair-1 regather settles during att(0,1)
                for _ in att_gen(0, j):
                    pump(QUOTA)
                work.append(wo_cols_gen(0, j, use_act=True))
            flush_chunk(5)
            for j in range(NQC):                      # b1 attention
                if j == 1:
                    flush_chunk(7)
                for _ in att_gen(1, j):
                    pump(QUOTA)
                work.append(wo_cols_gen(1, j, use_act=False))
            flush_all()
    nc.compile()
    return nc


def prep_inputs(inputs, cfg):
    B, L, H, D = cfg["B"], cfg["L"], cfg["H"], cfg["D"]
    HID = H * D
    BT = B * L
    F16 = np.float16
    x = np.asarray(inputs["x"], np.float32)
    Wqkv = np.asarray(inputs["Wqkv"], np.float32)
    Wo = np.asarray(inputs["Wo"], np.float32)
    qw = np.asarray(inputs["q_norm_w"], np.float32)
    kw = np.asarray(inputs["k_norm_w"], np.float32)
    cos = np.asarray(inputs["cos"], np.float32)[:L]
    sin = np.asarray(inputs["sin"], np.float32)[:L]
    d2 = D // 2

    xT = np.ascontiguousarray(x.reshape(BT, HID).T).astype(F16)
    # rope tables, gammas folded: t1 = A*gA*cs - B*gB*sn; t2 = A*gA*sn + B*gB*cs
    ct = np.tile(cos.T, (4, B))                      # (128, BT)
    st_ = np.tile(sin.T, (4, B))
    gA = np.zeros((128, 1), np.float32)
    gB = np.zeros((128, 1), np.float32)
    for m, w in enumerate([qw, qw, kw, kw]):
        rows = np.arange(32) + 32 * m
        gA[rows, 0] = w[:d2]
        gB[rows, 0] = w[d2:]
    ropet = np.stack([gA * ct, gA * st_, gB * ct, gB * st_], axis=1)
    ropet_d = np.ascontiguousarray(ropet).astype(F16)
    ki = np.arange(128)[:, None]
    jj = np.arange(128)[None, :]
    tri_d = ((jj < ki) * -240.0).astype(F16)   # -C on strict upper triangle of S^T
    grp = np.arange(128) // 32
    M_d = ((grp[:, None] == grp[None, :]).astype(np.float32) / D).astype(F16)
    id_d = np.eye(128, dtype=np.float32).astype(F16)
    c_bias = float(np.sqrt(D) * max(np.abs(qw).max() * np.abs(kw).max(), 1e-6))

    hpc = H // N_CORES
    in_maps = []
    for c in range(N_CORES):
        h0 = hpc * c
        h1 = h0 + 1
        d32 = np.arange(d2)
        Acols = np.r_[h0 * D + d32, h1 * D + d32,
                      HID + h0 * D + d32, HID + h1 * D + d32]
        Bcols = Acols + d2
        Ccols = np.r_[2 * HID + h0 * D + np.arange(D),
                      2 * HID + h1 * D + np.arange(D)]
        w_c = np.ascontiguousarray(
            Wqkv[:, np.r_[Acols, Bcols, Ccols]]).astype(F16)
        wo_c = np.ascontiguousarray(
            Wo[128 * c:128 * (c + 1), :]).astype(F16)
        in_maps.append(dict(xT=xT, wqkv=w_c, wo=wo_c, ropet_d=ropet_d,
                            tri_d=tri_d, M_d=M_d, id_d=id_d))
    return in_maps, c_bias


def gather_output(results, cfg):
    B, L, H, D = cfg["B"], cfg["L"], cfg["H"], cfg["D"]
    HID = H * D
    acc = np.zeros((HID, B * L), np.float32)
    for r in results:
        acc += r["outT"].astype(np.float32)
    return np.ascontiguousarray(acc.T).reshape(B, L, HID).astype(np.float32)


def kernel(**inputs):
    in_maps, c_bias = prep_inputs(inputs, CFG)
    nc = build_program(CFG, c_bias)
    res = bass_utils.run_bass_kernel_spmd(nc, in_maps,
                                          core_ids=list(range(N_CORES)))
    return gather_output(res.results, CFG)


# revision 4
# speedup vs baseline: 1.0116x; 1.0116x over previous
"""Causal self-attention (B=2, L=2048, H=16, D=64) head-sharded over 8 TRN2 cores.

v4: fp16, generator-scheduled single-pass program per core (heads {2c, 2c+1}):
  - fp16 everywhere except psum accumulation (f32).
  - emission is driven round-by-round: attention rounds yield between the
    S^T matmul/exp and the P@V consume; a worker queue of projection / Wo
    generators is pumped into each gap so PE never idles on the ACT exp.
  - rmsnorm: stats matmul with block-diagonal 1/D lhsT broadcasts var to all
    128 partitions; rstd = exp(-0.5*ln(var+eps)); gammas folded into 4 rope
    tables host-side; An/Bn/sq read the psum directly (no staging copies).
  - attention: S^T per 128-k-block; exp with constant bias; P@V token-major
    (out[q, d] per 128-q-block with ones-column rowsum); normalization via
    per-partition tensor_scalar; transpose to d-major via identity matmul.
  - Wo row-shard: each core emits a full-size fp16 partial out^T; host sums.
"""

from collections import deque

import numpy as np

import concourse.bacc as bacc
import concourse.bass as bass
import concourse.mybir as mybir
import concourse.tile as tile
from concourse import bass_utils
from concourse.hw_specs import get_activation_tables

F32 = mybir.dt.float32
FP16 = mybir.dt.float16

CFG = dict(B=2, L=2048, H=16, D=64, EPS=1e-6)
N_CORES = 8

TOKCH = 512   # token chunk for QKV projection
QCH = 512     # attention q chunk
KBLK = 128    # attention k block

ACT_TABLE = "natural_log_exp_and_others"


def _pin_act_table(nc):
    try:
        tabs = get_activation_tables(nc.m.arch)
    except Exception:
        return
    if ACT_TABLE in tabs:
        mine = {mybir.ActivationFunctionType.Exp,
                mybir.ActivationFunctionType.Ln,
                mybir.ActivationFunctionType.Copy}
        for k in tabs:
            if k != ACT_TABLE:
                tabs[k] = tabs[k] - mine


def build_program(cfg, c_bias, debug=False):
    B, L, H, D = cfg["B"], cfg["L"], cfg["H"], cfg["D"]
    HID = H * D
    BT = B * L
    NHID = HID // 128           # hidden (contraction) chunks (8)
    NQC = L // QCH              # q chunks per batch (4)
    NKB = L // KBLK             # k blocks per batch (16)
    CPB = L // TOKCH            # chunks per batch (4)
    scale = 1.0 / float(np.sqrt(D))
    Exp = mybir.ActivationFunctionType.Exp
    Ln = mybir.ActivationFunctionType.Ln

    nc = bacc.Bacc("TRN2", target_bir_lowering=False, debug=False,
                   num_devices=N_CORES)
    _pin_act_table(nc)

    xT = nc.dram_tensor("xT", [HID, BT], FP16, kind="ExternalInput").ap()
    wqkv = nc.dram_tensor("wqkv", [HID, 384], FP16, kind="ExternalInput").ap()
    wo = nc.dram_tensor("wo", [128, HID], FP16, kind="ExternalInput").ap()
    ropet_d = nc.dram_tensor("ropet_d", [128, 4, BT], FP16,
                             kind="ExternalInput").ap()
    tri_d = nc.dram_tensor("tri_d", [128, 128], FP16, kind="ExternalInput").ap()
    M_d = nc.dram_tensor("M_d", [128, 128], FP16, kind="ExternalInput").ap()
    id_d = nc.dram_tensor("id_d", [128, 128], FP16, kind="ExternalInput").ap()
    outT = nc.dram_tensor("outT", [HID, BT], FP16, kind="ExternalOutput").ap()

    with tile.TileContext(nc) as tc:
        with tc.tile_pool(name="const", bufs=1) as const, \
             tc.tile_pool(name="big", bufs=1) as big, \
             tc.tile_pool(name="p1", bufs=2) as p1, \
             tc.tile_pool(name="stps", bufs=2, space="PSUM") as stps, \
             tc.tile_pool(name="auxps", bufs=2, space="PSUM") as auxps, \
             tc.tile_pool(name="pvps", bufs=1, space="PSUM") as pvps:

            w_sbk = [const.tile([128, 384], FP16, name=f'w{k}')
                     for k in range(NHID)]
            ropet = const.tile([128, 4, BT], FP16)   # csA, snA, csB, snB
            wo_sb = const.tile([128, HID], FP16)
            tri_sb = const.tile([128, 128], FP16)
            M_sb = const.tile([128, 128], FP16)
            id_sb = const.tile([128, 128], FP16)
            eps_sb = const.tile([128, 1], F32)
            cb_sb = const.tile([128, 1], F32)
            zcol = const.tile([1, 128], FP16)
            zrow = const.tile([1, 512], FP16)

            QK = big.tile([128, 2 * BT], FP16)     # [:, 0:BT]=Q^T, [BT:]=K^T
            Vall = big.tile([128, NKB * B, 130], FP16)
            attnT = [big.tile([128, L], FP16, name=f"attnT{b}")
                     for b in range(B)]

            def emit_consts():
                nc.vector.memset(zcol, 0.0)
                nc.vector.memset(zrow, 0.0)
                nc.vector.memset(eps_sb, float(cfg["EPS"]))
                nc.vector.memset(cb_sb, -float(c_bias))
                nc.gpsimd.memset(Vall[:, :, 64:65], 1.0)
                nc.gpsimd.memset(Vall[:, :, 129:130], 1.0)
                nc.sync.dma_start(out=M_sb, in_=M_d)
                nc.sync.dma_start(out=id_sb, in_=id_d)
                nc.sync.dma_start(out=tri_sb, in_=tri_d)

            def emit_xload_split(t):
                x_sb = p1.tile([128, NHID, 2 * TOKCH], FP16, tag="x", bufs=2,
                               name="x_sb")
                for k in range(NHID):
                    nc.sync.dma_start(
                        out=x_sb[:, k, 0:TOKCH],
                        in_=xT[128 * k:128 * (k + 1), t * TOKCH:(t + 1) * TOKCH])
                return x_sb

            def emit_xload_second(x_sb, t):
                nc.sync.dma_start(
                    out=x_sb[:, :, TOKCH:2 * TOKCH],
                    in_=xT[:, t * TOKCH:(t + 1) * TOKCH].rearrange(
                        "(k p) t -> p k t", p=128))

            def emit_xload(pair):
                x_sb = p1.tile([128, NHID, 2 * TOKCH], FP16, tag="x", bufs=2,
                               name="x_sb")
                span = slice(pair * 2 * TOKCH, (pair + 1) * 2 * TOKCH)
                nc.sync.dma_start(
                    out=x_sb,
                    in_=xT[:, span].rearrange("(k p) t -> p k t", p=128))
                return x_sb

            t1big = {}
            t2big = {}
            x_tiles = {}

            def chunk_gen(t):
                """Projection + rmsnorm + rope for token chunk t.
                Yields PE-row counts between steps."""
                if t % 2 == 0 and t >= 4:
                    x_tiles[t // 2] = emit_xload(t // 2)
                x_sb = x_tiles[t // 2]
                xoff = (t % 2) * TOKCH
                xs = slice(xoff, xoff + TOKCH)
                blk0 = t * (TOKCH // KBLK)
                A_ps = auxps.tile([128, TOKCH], F32, tag="aux", name="A_ps")
                B_ps = auxps.tile([128, TOKCH], F32, tag="aux", name="B_ps")
                for m, ps in enumerate((A_ps, B_ps)):
                    for k in range(NHID):
                        nc.tensor.matmul(
                            ps, w_sbk[k][:, 128 * m:128 * (m + 1)],
                            x_sb[:, k, xs],
                            start=(k == 0), stop=(k == NHID - 1),
                            skip_group_check=True)
                        if k == 3:
                            yield 4 * TOKCH
                    yield 4 * TOKCH
                # stage A/B to sbuf (frees the aux psum slots), then stats
                A_sb = p1.tile([128, TOKCH], FP16, tag="A", bufs=3, name="A_sb")
                B_sb = p1.tile([128, TOKCH], FP16, tag="B", bufs=3, name="B_sb")
                nc.vector.tensor_copy(A_sb, A_ps)
                nc.vector.tensor_copy(B_sb, B_ps)
                sqA = p1.tile([128, TOKCH], FP16, tag="sqA", bufs=3, name="sqA")
                sqB = p1.tile([128, TOKCH], FP16, tag="sqB", bufs=3, name="sqB")
                An = p1.tile([128, TOKCH], FP16, tag="An", bufs=3, name="An")
                Bn = p1.tile([128, TOKCH], FP16, tag="Bn", bufs=3, name="Bn")
                nc.vector.tensor_mul(sqA, A_sb, A_sb)
                nc.vector.tensor_mul(sqB, B_sb, B_sb)
                var_ps = auxps.tile([128, TOKCH], F32, tag="aux", name="var_ps")
                nc.tensor.matmul(var_ps, M_sb, sqA, start=True, stop=False,
                                 skip_group_check=True)
                nc.tensor.matmul(var_ps, M_sb, sqB, start=False, stop=True,
                                 skip_group_check=True)
                lnv = p1.tile([128, TOKCH], F32, tag="lnv", name="lnv")
                nc.scalar.activation(lnv, var_ps, Ln, bias=eps_sb)
                rstd = p1.tile([128, TOKCH], FP16, tag="rstd", bufs=3, name="rstd")
                nc.scalar.activation(rstd, lnv, Exp, scale=-0.5)
                yield 2 * TOKCH
                # V projection (token-major)
                v_ps = auxps.tile([128, 4, 128], F32, tag="aux", name="v_ps")
                for blk in range(4):
                    for k in range(NHID):
                        nc.tensor.matmul(
                            v_ps[:, blk, :],
                            x_sb[:, k, xoff + 128 * blk: xoff + 128 * (blk + 1)],
                            w_sbk[k][:, 256:384],
                            start=(k == 0), stop=(k == NHID - 1),
                            skip_group_check=True)
                    if blk == 1:
                        yield 2 * 8 * 128
                nc.vector.tensor_copy(
                    Vall[:, blk0:blk0 + 4, :].rearrange(
                        "p c (a d) -> p c a d", a=2)[:, :, :, 0:64],
                    v_ps.rearrange("p c (a d) -> p c a d", a=2))
                nc.vector.tensor_mul(An, A_sb, rstd)
                nc.gpsimd.tensor_mul(Bn, B_sb, rstd)
                yield 2 * 8 * 128
                # rope
                pair = t // 2
                if pair not in t1big:
                    t1big[pair] = p1.tile([128, 2 * TOKCH], FP16, tag="t1",
                                          bufs=3, name="t1big")
                    t2big[pair] = p1.tile([128, 2 * TOKCH], FP16, tag="t2",
                                          bufs=3, name="t2big")
                t1 = t1big[pair][:, xs]
                t2 = t2big[pair][:, xs]
                ts_ = slice(t * TOKCH, (t + 1) * TOKCH)
                csA = ropet[:, 0, ts_]
                snA = ropet[:, 1, ts_]
                csB = ropet[:, 2, ts_]
                snB = ropet[:, 3, ts_]
                ta = p1.tile([128, TOKCH], FP16, tag="ta", name="ta")
                tb = p1.tile([128, TOKCH], FP16, tag="tb", name="tb")
                nc.vector.tensor_mul(ta, An, csA)
                nc.gpsimd.tensor_mul(tb, Bn, snB)
                nc.gpsimd.tensor_sub(t1, ta, tb)
                tc_ = p1.tile([128, TOKCH], FP16, tag="tc", name="tc_")
                td = p1.tile([128, TOKCH], FP16, tag="td", name="td")
                nc.vector.tensor_mul(tc_, An, snA)
                nc.gpsimd.tensor_mul(td, Bn, csB)
                nc.gpsimd.tensor_add(t2, tc_, td)
                if t % 2 == 1:
                    base = pair * 2 * TOKCH
                    for src, half in ((t1big[pair], 0), (t2big[pair], 1)):
                        for g in range(4):      # [q1h0|q1h1|k1h0|k1h1]
                            qk = g // 2
                            h = g % 2
                            dst = QK[64 * h + 32 * half:64 * h + 32 * (half + 1),
                                     qk * BT + base: qk * BT + base + 2 * TOKCH]
                            nc.sync.dma_start(
                                out=dst, in_=src[32 * g:32 * (g + 1), :])
                yield 0

            def att_gen(b, j):
                """Attention for q chunk j of batch b; yields between the
                st/exp emit and the (delayed one round) pv emit."""
                nkb = 4 * (j + 1)
                qbase = b * L + j * QCH
                pv = pvps.tile([128, 4, 2, 128], F32, tag="pv", name="pv")
                # zero the pv banks so every (qq, h) accumulation group can
                # use pure accumulates (start=False): psum start=True zeroes
                # whole 2KB regions, so per-group starts within a shared
                # bank would clobber siblings. Each group is padded to 128
                # f32 so no matmul output crosses a bank boundary.
                pvf = pv.rearrange("p a b c -> p (a b c)")
                nc.tensor.matmul(pvf[:, 0:512], zcol, zrow,
                                 start=True, stop=False, skip_group_check=True)
                nc.tensor.matmul(pvf[:, 512:1024], zcol, zrow,
                                 start=True, stop=False, skip_group_check=True)
                fin_q = deque()

                def consume(i, pexp):
                    s_off = KBLK * i - QCH * j
                    diag = s_off >= 0
                    qq0 = max(0, i - 4 * j)
                    for qq in list(range(qq0 + 1, 4)) + [qq0]:
                        qsl = slice(128 * qq, 128 * (qq + 1))
                        qb = 4 * j + qq
                        for h in range(2):
                            nc.tensor.matmul(
                                pv[:, qq, h, 0:65],
                                pexp[:, h, qsl],
                                Vall[:, b * NKB + i, 65 * h:65 * (h + 1)],
                                start=False, stop=(i == qb),
                                skip_group_check=True)
                    if diag:
                        qq = i - 4 * j
                        qb = 4 * j + qq
                        rs = p1.tile([128, 2, 1], F32, tag="rs", bufs=8,
                                     name="rs")
                        with nc.allow_low_precision(reason="rowsum recip"):
                            nc.vector.reciprocal(rs, pv[:, qq, :, 64:65])
                        attn_sb = p1.tile([128, 128], FP16, tag="attn",
                                          bufs=8, name="attn_sb")
                        nc.vector.tensor_scalar_mul(
                            attn_sb[:, 0:64], pv[:, qq, 0, 0:64], rs[:, 0, :])
                        nc.vector.tensor_scalar_mul(
                            attn_sb[:, 64:128], pv[:, qq, 1, 0:64], rs[:, 1, :])
                        tr = auxps.tile([128, 128], F32, tag="aux", name="tr")
                        nc.tensor.matmul(tr, attn_sb, id_sb,
                                         start=True, stop=True,
                                         skip_group_check=True)
                        nc.vector.tensor_copy(
                            attnT[b][:, 128 * qb:128 * (qb + 1)], tr)

                pending = deque()
                for i in range(nkb):
                    s_off = KBLK * i - QCH * j
                    diag = s_off >= 0
                    sp = slice(max(s_off, 0), QCH)
                    st = stps.tile([128, 2, QCH], F32, tag="st", name="st")
                    for h in range(2):
                        nc.tensor.matmul(
                            st[:, h, sp],
                            QK[64 * h:64 * (h + 1),
                               BT + b * L + KBLK * i: BT + b * L + KBLK * (i + 1)],
                            QK[64 * h:64 * (h + 1), qbase + sp.start:qbase + QCH],
                            start=True, stop=not diag, skip_group_check=True)
                        if diag:
                            # add -C to the strict upper triangle of the
                            # diagonal 128-block so exp gives ~0 there
                            nc.tensor.matmul(
                                st[:, h, sp.start:sp.start + KBLK],
                                id_sb, tri_sb,
                                start=False, stop=True, skip_group_check=True)
                    pexp = p1.tile([128, 2, QCH], FP16, tag="pexp", bufs=9,
                                   name="pexp")
                    nc.scalar.activation(
                        pexp[:, :, sp], st[:, :, sp],
                        Exp, bias=cb_sb, scale=scale)
                    yield
                    pending.append((i, pexp))
                    if len(pending) > 7:
                        consume(*pending.popleft())
                    yield
                while pending:
                    consume(*pending.popleft())

            wo_cp = [0]
            ob1 = {}

            def wo_cols_gen(b, jj, use_act=True):     # jj-major
                js = slice(jj * QCH, (jj + 1) * QCH)
                jl = slice((jj % 2) * QCH, (jj % 2 + 1) * QCH)
                for o in range(NHID):
                    if (b, o) not in ob1:
                        ob1[(b, o)] = p1.tile([128, 2 * QCH], FP16,
                                              tag=f"ob{b}", bufs=NHID,
                                              name=f"ob{b}_{o}")
                    ops = auxps.tile([128, QCH], F32, tag="aux", name="ops")
                    nc.tensor.matmul(ops, wo_sb[:, 128 * o:128 * (o + 1)],
                                     attnT[b][:, js],
                                     start=True, stop=True,
                                     skip_group_check=True)
                    if use_act and wo_cp[0] % 3 == 2:
                        nc.scalar.copy(ob1[(b, o)][:, jl], ops)
                    else:
                        nc.vector.tensor_copy(ob1[(b, o)][:, jl], ops)
                    wo_cp[0] += 1
                    yield QCH
                if jj % 2 == 1:               # store a half per o
                    hs = slice((jj - 1) * QCH, (jj + 1) * QCH)
                    for o in range(NHID):
                        nc.sync.dma_start(
                            out=outT[128 * o:128 * (o + 1),
                                     b * L + hs.start:b * L + hs.stop],
                            in_=ob1[(b, o)])

            # ---------- scheduler ----------
            work = deque()
            done_chunks = set()

            def wrap_chunk(t):
                def g():
                    for r in chunk_gen(t):
                        yield r
                    done_chunks.add(t)
                return g()

            def pump(rows):
                while rows > 0 and work:
                    try:
                        rows -= next(work[0])
                    except StopIteration:
                        work.popleft()

            def flush_chunk(t):
                while t not in done_chunks and work:
                    try:
                        next(work[0])
                    except StopIteration:
                        work.popleft()

            def flush_all():
                while work:
                    try:
                        next(work[0])
                    except StopIteration:
                        work.popleft()

            QUOTA = 500          # PE rows pumped per attention yield

            # ---------- emission schedule ----------
            for k in range(NHID):
                nc.sync.dma_start(out=w_sbk[k],
                                  in_=wqkv[128 * k:128 * (k + 1), :])
            x_tiles[0] = emit_xload_split(0)
            emit_consts()
            nc.sync.dma_start(out=ropet[:, :, 0:2 * TOKCH],
                              in_=ropet_d[:, :, 0:2 * TOKCH])
            emit_xload_second(x_tiles[0], 1)
            x_tiles[1] = emit_xload(1)        # prefetch chunks 2-3
            for _ in chunk_gen(0):
                pass
            nc.sync.dma_start(out=ropet[:, :, 2 * TOKCH:],
                              in_=ropet_d[:, :, 2 * TOKCH:])
            for _ in chunk_gen(1):
                pass
            nc.sync.dma_start(out=wo_sb, in_=wo)
            work.extend([wrap_chunk(2), wrap_chunk(3), wrap_chunk(4),
                         wrap_chunk(5), wrap_chunk(6), wrap_chunk(7)])
            for j in range(NQC):                      # b0 attention
                if j == 1:
                    flush_chunk(3)    # pair-1 regather settles during att(0,1)
                for _ in att_gen(0, j):
                    pump(QUOTA)
                work.append(wo_cols_gen(0, j, use_act=True))
            flush_chunk(5)
            for j in range(NQC):                      # b1 attention
                if j == 1:
                    flush_chunk(7)
                for _ in att_gen(1, j):
                    pump(QUOTA)
                work.append(wo_cols_gen(1, j, use_act=False))
            flush_all()
    nc.compile()
    return nc


def prep_inputs(inputs, cfg):
    B, L, H, D = cfg["B"], cfg["L"], cfg["H"], cfg["D"]
    HID = H * D
    BT = B * L
    F16 = np.float16
    x = np.asarray(inputs["x"], np.float32)
    Wqkv = np.asarray(inputs["Wqkv"], np.float32)
    Wo = np.asarray(inputs["Wo"], np.float32)
    qw = np.asarray(inputs["q_norm_w"], np.float32)
    kw = np.asarray(inputs["k_norm_w"], np.float32)
    cos = np.asarray(inputs["cos"], np.float32)[:L]
    sin = np.asarray(inputs["sin"], np.float32)[:L]
    d2 = D // 2

    xT = np.ascontiguousarray(x.reshape(BT, HID).T).astype(F16)
    # rope tables, gammas folded: t1 = A*gA*cs - B*gB*sn; t2 = A*gA*sn + B*gB*cs
    ct = np.tile(cos.T, (4, B))                      # (128, BT)
    st_ = np.tile(sin.T, (4, B))
    gA = np.zeros((128, 1), np.float32)
    gB = np.zeros((128, 1), np.float32)
    for m, w in enumerate([qw, qw, kw, kw]):
        rows = np.arange(32) + 32 * m
        gA[rows, 0] = w[:d2]
        gB[rows, 0] = w[d2:]
    ropet = np.stack([gA * ct, gA * st_, gB * ct, gB * st_], axis=1)
    ropet_d = np.ascontiguousarray(ropet).astype(F16)
    ki = np.arange(128)[:, None]
    jj = np.arange(128)[None, :]
    tri_d = ((jj < ki) * -240.0).astype(F16)   # -C on strict upper triangle of S^T
    grp = np.arange(128) // 32
    M_d = ((grp[:, None] == grp[None, :]).astype(np.float32) / D).astype(F16)
    id_d = np.eye(128, dtype=np.float32).astype(F16)
    c_bias = float(np.sqrt(D) * max(np.abs(qw).max() * np.abs(kw).max(), 1e-6))

    hpc = H // N_CORES
    in_maps = []
    for c in range(N_CORES):
        h0 = hpc * c
        h1 = h0 + 1
        d32 = np.arange(d2)
        Acols = np.r_[h0 * D + d32, h1 * D + d32,
                      HID + h0 * D + d32, HID + h1 * D + d32]
        Bcols = Acols + d2
        Ccols = np.r_[2 * HID + h0 * D + np.arange(D),
                      2 * HID + h1 * D + np.arange(D)]
        w_c = np.ascontiguousarray(
            Wqkv[:, np.r_[Acols, Bcols, Ccols]]).astype(F16)
        wo_c = np.ascontiguousarray(
            Wo[128 * c:128 * (c + 1), :]).astype(F16)
        in_maps.append(dict(xT=xT, wqkv=w_c, wo=wo_c, ropet_d=ropet_d,
                            tri_d=tri_d, M_d=M_d, id_d=id_d))
    return in_maps, c_bias


def gather_output(results, cfg):
    B, L, H, D = cfg["B"], cfg["L"], cfg["H"], cfg["D"]
    HID = H * D
    acc = np.zeros((HID, B * L), np.float32)
    for r in results:
        acc += r["outT"].astype(np.float32)
    return np.ascontiguousarray(acc.T).reshape(B, L, HID).astype(np.float32)


def kernel(**inputs):
    in_maps, c_bias = prep_inputs(inputs, CFG)
    nc = build_program(CFG, c_bias)
    res = bass_utils.run_bass_kernel_spmd(nc, in_maps,
                                          core_ids=list(range(N_CORES)))
    return gather_output(res.results, CFG)
